# revision 9
# baseline (speedup 1.0000x reference)
"""Block-sparse flash attention (Phi-3-small pattern) on 8 Trainium2 cores.

Problem: S=2048 tokens, 32 query heads, 8 KV heads (GQA x4), D=128,
sparse_block_size=64, local_blocks=16, vert_stride=8, per-head vertical
offset (homo_head=False).

Sharding: tensor-parallel over heads. Core r owns contiguous heads
[4r, 4r+4), which all share GQA KV head r.

Per-head block mask (head h, c = (7-h) % 8):
  block (qb, kb) active iff qb >= kb and (qb-kb < 16 or kb % 8 == c)
Decomposition used here (verified exact vs reference on host):
  - LOCAL pass, k-tile kt covers kbs {2kt, 2kt+1}: q in [128kt, 128kt+1088)
      * causal triangle inside the diagonal 128x128 block
      * zero k-rows [0:64) for the last 64 q cols (qb-kb == 16 corner)
  - TAIL pass: the two vertical kbs {c, c+8} gathered on the host into one
    128-row k-tile; q in [1024, 2048) with a host-built 0/1 mask
    (rows 0:64 active for q >= 1024+64c, rows 64:128 for q >= 1536+64c).

Kernel math (scores bounded, so softmax without max-subtraction is exact
to ~1e-6):  scoresT[k,q] = (K^T)^T.T @ Q^T on PE (contraction D=128 on
partitions, so PV needs no transposes), E^T = exp(SCALE*scoresT) on ACT
(bf16), out^T accumulated in PSUM over k-tiles, rowsum via ones-matmul,
final PE transpose + per-partition 1/rowsum scale.

All per-head pattern differences are input DATA, so the single SPMD
program is identical on all 8 cores.
"""

import sys
from contextlib import ExitStack

import numpy as np

for _p in ("/opt/trn_rl_repo", "/root/.axon_site/_ro/trn_rl_repo"):
    if _p not in sys.path:
        sys.path.append(_p)

import ml_dtypes

import concourse.bass as bass
import concourse.bacc as bacc
import concourse.mybir as mybir
import concourse.tile as tile
from concourse.bass_utils import run_bass_kernel_spmd

S = 2048
D = 128
H = 32
HKV = 8
NCORES = 8
NH = H // NCORES          # heads per core = 4
SCALE = 0.08838834764831845
NKT = S // 128            # 16 k-tiles of 128 rows
SPAN = 1088               # local window cols per k-tile (17 blocks of 64)
HALF = 1024
WIN = 512                 # PSUM bank window

BF16 = mybir.dt.bfloat16
F32 = mybir.dt.float32
NPBF16 = ml_dtypes.bfloat16


def _chunks_for(kt, half):
    """512-aligned chunks of the local span of k-tile kt inside a q-half."""
    lo = max(128 * kt, HALF * half)
    hi = min(128 * kt + SPAN, S, HALF * half + HALF)
    res = []
    a = lo
    while a < hi:
        b = min((a // WIN + 1) * WIN, hi)
        res.append((a, b))
        a = b
    return res


def build_program(loop_n=1):
    nc = bacc.Bacc("TRN2", target_bir_lowering=False, debug=False)
    qT = nc.dram_tensor("qT", [NH, 128, S], BF16, kind="ExternalInput").ap()
    kT = nc.dram_tensor("kT", [128, S], BF16, kind="ExternalInput").ap()
    vR = nc.dram_tensor("vR", [128, S], BF16, kind="ExternalInput").ap()
    kvT = nc.dram_tensor("kvT", [NH, 128, 128], BF16, kind="ExternalInput").ap()
    vv = nc.dram_tensor("vv", [NH, 128, 128], BF16, kind="ExternalInput").ap()
    tb = nc.dram_tensor("tbias", [NH, 2, HALF], BF16, kind="ExternalInput").ap()
    trb = nc.dram_tensor("tribias", [128, 128], BF16, kind="ExternalInput").ap()
    idb = nc.dram_tensor("identb", [128, 128], BF16, kind="ExternalInput").ap()
    u2 = nc.dram_tensor("u2", [2, 128], BF16, kind="ExternalInput").ap()
    cb = nc.dram_tensor("cb", [2, 64], BF16, kind="ExternalInput").ap()
    idn = nc.dram_tensor("ident", [128, 128], F32, kind="ExternalInput").ap()
    out = nc.dram_tensor("out", [S, NH * 128], F32, kind="ExternalOutput").ap()

    Exp = mybir.ActivationFunctionType.Exp

    with tile.TileContext(nc) as tc, ExitStack() as ctx:
        const = ctx.enter_context(tc.tile_pool(name="const", bufs=1))
        perhead = ctx.enter_context(tc.tile_pool(name="perhead", bufs=2))
        eTp = ctx.enter_context(tc.tile_pool(name="eT", bufs=4))
        osb = ctx.enter_context(tc.tile_pool(name="osb", bufs=3))
        smal = ctx.enter_context(tc.tile_pool(name="small", bufs=2))
        scp = ctx.enter_context(tc.tile_pool(name="scores", bufs=3, space="PSUM"))
        otp = ctx.enter_context(tc.tile_pool(name="outT", bufs=2, space="PSUM"))
        rsp = ctx.enter_context(tc.tile_pool(name="rs", bufs=2, space="PSUM"))
        tpp = ctx.enter_context(tc.tile_pool(name="tp", bufs=1, space="PSUM"))
        drp = ctx.enter_context(tc.tile_pool(name="dram", bufs=2, space="DRAM"))

        kT_sb = const.tile([128, S], BF16, tag="kT")
        nc.sync.dma_start(kT_sb[:], kT[:])
        v_sb = const.tile([128, S], BF16, tag="v")
        nc.sync.dma_start(v_sb[:], vR[:])
        trb_sb = const.tile([128, 128], BF16, tag="trb")
        nc.sync.dma_start(trb_sb[:], trb[:])
        idb_sb = const.tile([128, 128], BF16, tag="idb")
        nc.sync.dma_start(idb_sb[:], idb[:])
        u2_sb = const.tile([2, 128], BF16, tag="u2")
        nc.sync.dma_start(u2_sb[:], u2[:])
        id_sb = const.tile([128, 128], F32, tag="id")
        nc.sync.dma_start(id_sb[:], idn[:])
        ones_sb = const.tile([128, 1], BF16, tag="ones")
        nc.vector.memset(ones_sb[:], 1.0)
        cb_sb = const.tile([2, 64], BF16, tag="cb")
        nc.sync.dma_start(cb_sb[:], cb[:])

        loop_cm = (tc.For_i(0, loop_n, 1,
                            hint_engines=(mybir.EngineType.PE,
                                          mybir.EngineType.Activation,
                                          mybir.EngineType.DVE,
                                          mybir.EngineType.SP))
                   if loop_n > 1 else None)
        if loop_cm is not None:
            loop_cm.__enter__()
        for h in range(NH):
            qT_sb = perhead.tile([128, S], BF16, tag="qT")
            nc.sync.dma_start(qT_sb[:], qT[h])
            kvT_sb = perhead.tile([128, 128], BF16, tag="kvT")
            nc.sync.dma_start(kvT_sb[:], kvT[h])
            vv_sb = perhead.tile([128, 128], BF16, tag="vv")
            nc.sync.dma_start(vv_sb[:], vv[h])
            tb_sb = perhead.tile([2, HALF], BF16, tag="tb")
            nc.sync.dma_start(tb_sb[:], tb[h])

            for half in (0, 1):
                half_lo = HALF * half
                half_hi = half_lo + HALF

                # ---- plan the work for this (head, half) ----
                # step = (kind, kt, a, b, parts); parts = [(lo, hi), ...]
                # split at the coverage boundary so each PV/RS matmul's
                # PSUM elements are uniformly fresh or accumulating.
                steps = []
                for kt in range(NKT):
                    cov = 0 if kt == 0 else min(1088 + 128 * (kt - 1), S)
                    cov = min(max(cov, half_lo), half_hi)
                    for (a, b) in _chunks_for(kt, half):
                        if cov <= a:
                            parts = [(a, b)]
                        elif cov >= b:
                            parts = [(a, b)]
                        else:
                            parts = [(a, cov), (cov, b)]
                        steps.append(("loc", kt, a, b, parts))
                if half == 1:
                    for (a, b) in ((1024, 1536), (1536, 2048)):
                        steps.append(("tail", -1, a, b, [(a, b)]))

                # last matmul per window (for stop=True)
                n_into_w = [0, 0]
                for (_, _, _, _, parts) in steps:
                    for (lo, hi) in parts:
                        n_into_w[(lo - half_lo) // WIN] += 1

                ow = [otp.tile([128, WIN], F32, tag="ow", name=f"ow{w}") for w in range(2)]
                rs = [rsp.tile([1, WIN], F32, tag="rs", name=f"rs{w}") for w in range(2)]
                w_started = [False, False]
                w_seen = [0, 0]

                # ---- emit ----
                for (kind, kt, a, b, parts) in steps:
                    n = b - a
                    sc = scp.tile([128, WIN], F32, tag="sc")
                    # which additive mask-bias matmul follows the QK matmul?
                    if kind == "loc":
                        lhs_qk = kT_sb[:, 128 * kt:128 * kt + 128]
                        has_tri = a == 128 * kt and kt // 8 == half
                        has_cor = kt <= 7 and b == 128 * kt + 1088
                        has_tail = False
                    else:
                        lhs_qk = kvT_sb[:]
                        has_tri = has_cor = False
                        has_tail = True
                    nbias = int(has_tri) + int(has_cor) + int(has_tail)
                    nc.tensor.matmul(sc[:, 0:n], lhs_qk, qT_sb[:, a:b],
                                     start=True, stop=nbias == 0)
                    left = nbias
                    if has_tri:
                        # causal triangle on the diagonal block: += tribias
                        left -= 1
                        nc.tensor.matmul(sc[:, 0:128], idb_sb[:], trb_sb[:],
                                         start=False, stop=left == 0)
                    if has_cor:
                        # qb-kb == 16 corner: += -1e5 on k rows [0:64)
                        left -= 1
                        rel = (128 * kt + 1024) - a
                        nc.tensor.matmul(sc[:, rel:rel + 64], u2_sb[:],
                                         cb_sb[:], start=False, stop=left == 0)
                    if has_tail:
                        # per-head vertical-stride cutoffs (rank-2 bias)
                        left -= 1
                        nc.tensor.matmul(sc[:, 0:n], u2_sb[:],
                                         tb_sb[:, a - HALF:b - HALF],
                                         start=False, stop=left == 0)
                    eT = eTp.tile([128, WIN], BF16, tag="eT")
                    nc.scalar.activation(eT[:, 0:n], sc[:, 0:n], Exp, scale=SCALE)
                    lhs_pv = v_sb[:, 128 * kt:128 * kt + 128] if kind == "loc" else vv_sb[:]

                    for (lo, hi) in parts:
                        w = (lo - half_lo) // WIN
                        wl = half_lo + WIN * w
                        st = not w_started[w]
                        w_started[w] = True
                        w_seen[w] += 1
                        sp = w_seen[w] == n_into_w[w]
                        nc.tensor.matmul(ow[w][:, lo - wl:hi - wl], lhs_pv,
                                         eT[:, lo - a:hi - a], start=st, stop=sp)
                        nc.tensor.matmul(rs[w][0:1, lo - wl:hi - wl],
                                         ones_sb[:, 0:1], eT[:, lo - a:hi - a],
                                         start=st, stop=sp)

                # ---- epilogue ----
                rs_row = smal.tile([1, HALF], F32, tag="rsrow")
                nc.vector.tensor_copy(rs_row[0:1, 0:WIN], rs[0][:])
                nc.vector.tensor_copy(rs_row[0:1, WIN:HALF], rs[1][:])
                dscr = drp.tile([1, HALF], F32, tag="scr")
                nc.sync.dma_start(dscr[:], rs_row[:])
                rsT = smal.tile([128, 8], F32, tag="rsT")
                nc.sync.dma_start(rsT[:],
                                  dscr[:].rearrange("a (t p) -> (a p) t", p=128))
                rcp = smal.tile([128, 8], F32, tag="rcp")
                nc.vector.reciprocal(rcp[:], rsT[:])

                for w in range(2):
                    ocp = osb.tile([128, WIN], F32, tag="ocp")
                    nc.vector.tensor_copy(ocp[:], ow[w][:])
                    for j in range(4):
                        col = 4 * w + j
                        tp = tpp.tile([128, 128], F32, tag="tp")
                        nc.tensor.transpose(tp[:], ocp[:, 128 * j:128 * j + 128],
                                            id_sb[:])
                        os_t = osb.tile([128, 128], F32, tag="os")
                        nc.vector.tensor_scalar_mul(os_t[:], tp[:],
                                                    rcp[:, col:col + 1])
                        q0 = half_lo + 128 * col
                        nc.sync.dma_start(out[q0:q0 + 128, 128 * h:128 * h + 128],
                                          os_t[:])
        if loop_cm is not None:
            loop_cm.__exit__(None, None, None)
    nc.compile()
    return nc


def make_core_inputs(query, key, value, core):
    """Host-side prep of one core's input map (bf16, pre-transposed/gathered)."""
    q3 = query.reshape(S, H, D)
    k3 = key.reshape(S, HKV, D)
    v3 = value.reshape(S, HKV, D)
    r = core
    K = k3[:, r, :]                     # [S, 128]
    V = v3[:, r, :]
    KT = np.ascontiguousarray(K.T)      # [128, S]
    vRe = np.ascontiguousarray(
        V.reshape(NKT, 128, D).transpose(1, 0, 2).reshape(128, S))

    NEG = np.float32(-100000.0)
    qT = np.empty((NH, 128, S), NPBF16)
    kvT = np.empty((NH, 128, 128), NPBF16)
    vv = np.empty((NH, 128, 128), NPBF16)
    tbias = np.zeros((NH, 2, HALF), NPBF16)
    for hl in range(NH):
        hg = NH * r + hl
        c = (7 - hg) % 8
        qT[hl] = q3[:, hg, :].T.astype(NPBF16)
        kvT[hl, :, 0:64] = KT[:, 64 * c:64 * c + 64].astype(NPBF16)
        kvT[hl, :, 64:128] = KT[:, 64 * (c + 8):64 * (c + 8) + 64].astype(NPBF16)
        vv[hl, 0:64, :] = V[64 * c:64 * c + 64, :].astype(NPBF16)
        vv[hl, 64:128, :] = V[64 * (c + 8):64 * (c + 8) + 64, :].astype(NPBF16)
        qq = np.arange(HALF)
        tbias[hl, 0, :] = np.where(qq < 64 * c, NEG, 0.0).astype(NPBF16)
        tbias[hl, 1, :] = np.where(qq < 512 + 64 * c, NEG, 0.0).astype(NPBF16)

    kk = np.arange(128)[:, None]
    qq = np.arange(128)[None, :]
    tribias = np.where(qq >= kk, 0.0, NEG).astype(NPBF16)
    u2 = np.zeros((2, 128), NPBF16)
    u2[0, 0:64] = 1.0
    u2[1, 64:128] = 1.0

    return {
        "qT": qT,
        "kT": KT.astype(NPBF16),
        "vR": vRe.astype(NPBF16),
        "kvT": kvT,
        "vv": vv,
        "tbias": tbias,
        "tribias": tribias,
        "identb": np.eye(128, dtype=NPBF16),
        "u2": u2,
        "cb": np.concatenate([np.full((1, 64), NEG, NPBF16),
                              np.zeros((1, 64), NPBF16)], axis=0),
        "ident": np.eye(128, dtype=np.float32),
    }


_PROGRAM = None


def _get_program():
    global _PROGRAM
    if _PROGRAM is None:
        _PROGRAM = build_program()
    return _PROGRAM


def run(query, key, value, trace=False):
    """Returns (output [S, H*D] f32, BassKernelResults)."""
    nc = _get_program()
    in_maps = [make_core_inputs(query, key, value, r) for r in range(NCORES)]
    br = run_bass_kernel_spmd(nc, in_maps, list(range(NCORES)), trace=trace)
    outp = np.hstack([br.results[r]["out"] for r in range(NCORES)])
    return outp, br


def kernel(query, key, value):
    outp, _ = run(np.asarray(query), np.asarray(key), np.asarray(value))
    return outp
